# revision 10
# baseline (speedup 1.0000x reference)
"""Trainium2 Bass kernel for nn_CustomANFIS (N=4096, D=128, R=256, O=64).

Math (reference):
  memb[n,r,d]  = exp(-(x[n,d]-c[r,d])^2 / (2 s[r,d]^2))
  str[n,r]     = prod_d memb = exp(-q[n,r]) with
                 q[n,r] = sum_d x^2[n,d]*A[d,r] + sum_d x[n,d]*B[d,r] + G[r],
                 A = 1/(2 s^2), B = -c/s^2, G = sum_d c^2/(2 s^2)
  den[n]       = sum_r str + 1e-8
  W[n,r,:]     = x[n,:] @ coeffs[r,:D,:] + coeffs[r,D,:]
  out          = softmax_j( (1/den) * sum_r str[n,r] * W[n,r,j] )

Device algorithm (data-parallel over N across 8 cores):
  1. strengths^T [r (2 part-tiles), n=512] via 2 fp32r accumulating matmuls
     (x^2 transposed comes precomputed from the host) + ACT exp
     (per-partition bias=-G), written as bf16.
  2. per n-tile: den column + bias consequent via one small matmul against
     [ones | ones | Cb] (cbo), emitted just before the n-tile's chunks so
     the stationary st_n load is shared.
  3. T[n, (j,d)] = sum_r sT[r,n] * C[r, (j,d)] in bf16, 8 chunks of 1024
     (8 j x 128 d) per n-tile, PSUM-accumulated over the 2 r K-tiles.
  4. prod[n, j, d] = X[n,d] * T[n,j,d] via per-chunk paths balanced over
     three engines:
       'A': ACT PSUM->SBUF bf16 copy, then DVE multiply (2x packed)
       'P': ACT copy, then GpSimd multiply
       'F': DVE multiply straight out of PSUM (no ACT copy)
     d-reduction: one tensor_reduce per j-half (or a pairwise bf16
     pyramid, TREE_MODE knob).
  5. logits = acc/den; softmax over j WITHOUT max-subtraction (logits are
     bounded by ~1.2, exp cannot overflow): exp(acc*scalec) with
     accum_out, reciprocal, scale-copy. Softmax of n-tile i is emitted
     inside n-tile i+1's chunk stream so the scalar queue never stalls.

DMA: inputs are spread over all five engine queues so the strengths
inputs and the first C chunks land in parallel (per-queue FIFO was the
head bottleneck); C chunks round-robin across queues.
"""

import numpy as np
import ml_dtypes

N, D, R, O = 4096, 128, 256, 64
NCORES = 8
NS = N // NCORES          # 512 rows per core
NT = NS // 128            # 4 n-tiles per core
RT = R // 128             # 2 r k-tiles
DJ = D * O                # 8192
CHUNK = 1024              # 2 PSUM banks per chunk
NCHUNK = DJ // CHUNK      # 8 chunks (8 j x 128 d each)
JPC = CHUNK // D          # 8 j per chunk
MM = 512                  # moving free dim per matmul

# per-chunk consumer path within each n-tile:
#   'A': ACT copy + DVE multiply   'P': ACT copy + GpSimd multiply
#   'F': DVE multiply from PSUM (no ACT)
PATHS = (
    ('A', 'P', 'A', 'A', 'P', 'F', 'A', 'P'),
    ('A', 'P', 'A', 'A', 'P', 'F', 'A', 'P'),
    ('A', 'P', 'A', 'A', 'P', 'F', 'A', 'P'),
    ('A', 'P', 'A', 'A', 'A', 'F', 'F', 'F'),
)
TREE_MODE = 'pyramid'     # 'reduce': one tensor_reduce per j-half (1x, slow)
                          # 'pyramid': pairwise bf16 tree to d=8 + reduce

_CACHE = {}
BF16 = ml_dtypes.bfloat16


def _build():
    import concourse.bass as bass
    import concourse.tile as tile
    from concourse import bacc, mybir

    f32 = mybir.dt.float32
    f32r = mybir.dt.float32r
    bf16 = mybir.dt.bfloat16
    AF = mybir.ActivationFunctionType
    ALU = mybir.AluOpType
    ts = bass.ts

    nc = bacc.Bacc(
        "TRN2", target_bir_lowering=False, debug=False, num_devices=NCORES
    )

    xt_d = nc.dram_tensor("xt", [D, NS], bf16, kind="ExternalInput").ap()
    x2t_d = nc.dram_tensor("x2t", [D, NS], bf16, kind="ExternalInput").ap()
    xn_d = nc.dram_tensor("xn", [128, NT * D], bf16, kind="ExternalInput").ap()
    a_d = nc.dram_tensor("a_p", [D, R], bf16, kind="ExternalInput").ap()
    b_d = nc.dram_tensor("b_p", [D, R], bf16, kind="ExternalInput").ap()
    ng_d = nc.dram_tensor("negg", [128, RT], f32, kind="ExternalInput").ap()
    # C in [128 r-low, rt, (j,d)] layout so one dma_start per chunk covers
    # both rule K-tiles with a partition-major access pattern.
    c_d = nc.dram_tensor("cflat", [128, RT * DJ], bf16, kind="ExternalInput").ap()
    cbo_d = nc.dram_tensor("cbo", [128, RT * (O + 2)], bf16, kind="ExternalInput").ap()
    out_d = nc.dram_tensor("out", [NS, O], f32, kind="ExternalOutput").ap()

    def r32(ap):
        return ap if ap.dtype == f32r else ap.bitcast(f32r)

    with tile.TileContext(nc) as tc:
        from contextlib import ExitStack

        with ExitStack() as ctx:
            konst = ctx.enter_context(tc.tile_pool(name="konst", bufs=1))
            cw = ctx.enter_context(tc.tile_pool(name="cw", bufs=1))
            stp = ctx.enter_context(tc.tile_pool(name="stp", bufs=1))
            prodp = ctx.enter_context(tc.tile_pool(name="prodp", bufs=3))
            small = ctx.enter_context(tc.tile_pool(name="small", bufs=4))
            psum = ctx.enter_context(tc.tile_pool(name="psum", bufs=2, space="PSUM"))

            xt_sb = konst.tile([D, NS], bf16)
            x2t_sb = konst.tile([D, NS], bf16)
            a_sb = konst.tile([D, R], bf16)
            b_sb = konst.tile([D, R], bf16)
            ng_sb = konst.tile([128, RT], f32)
            c_sb = cw.tile([128, RT * DJ], bf16)
            cbo_sb = cw.tile([128, RT * (O + 2)], bf16)
            xn_sb = konst.tile([128, NT * D], bf16)

            # --- input DMA spread: per-queue FIFO order is the latency
            # constraint, so strengths inputs go first on each queue, then
            # the C chunks round-robin.
            c_v = c_sb[:].rearrange("p (t f) -> p t f", t=RT)
            cd_v = c_d.rearrange("p (t f) -> p t f", t=RT)

            def c_dma(eng, c):
                csl = slice(c * CHUNK, (c + 1) * CHUNK)
                eng.dma_start(c_v[:, :, csl], cd_v[:, :, csl])

            # Only sync/scalar (HWDGE, ~0.6us first byte) and gpsimd (SWDGE)
            # can initiate DMAs. gpsimd's first DMA pays a ~6us Q7 ucode
            # library load, so nothing latency-critical rides gpsimd.
            nc.sync.dma_start(xt_sb[:], xt_d)
            nc.sync.dma_start(b_sb[:], b_d)
            nc.scalar.dma_start(x2t_sb[:], x2t_d)
            nc.scalar.dma_start(ng_sb[:], ng_d)
            nc.scalar.dma_start(a_sb[:], a_d)
            nc.scalar.dma_start(xn_sb[:], xn_d)
            nc.scalar.dma_start(cbo_sb[:], cbo_d)
            c_dma(nc.sync, 0)
            c_dma(nc.scalar, 1)
            c_dma(nc.sync, 2)
            c_dma(nc.scalar, 3)
            c_dma(nc.sync, 4)
            c_dma(nc.scalar, 5)
            c_dma(nc.sync, 6)
            c_dma(nc.gpsimd, 7)

            # warm the ACT exp table set (~2.7us load) during the DMA head
            dummy = konst.tile([128, 1], f32, name="dummy")
            nc.vector.memset(dummy[:], 0.0)
            dummy2 = konst.tile([128, 1], f32, name="dummy2")
            nc.scalar.activation(dummy2[:], dummy[:], AF.Exp)

            # ---- strengths^T: [r-tile partitions, n free], bf16
            st_tiles = []
            for rt in range(RT):
                sps = psum.tile([128, CHUNK], f32, tag="bank2", name=f"sps{rt}", bufs=4)
                nc.tensor.matmul(
                    sps[:, :NS], b_sb[:, ts(rt, 128)], xt_sb[:],
                    start=True, stop=False,
                )
                nc.tensor.matmul(
                    sps[:, :NS], a_sb[:, ts(rt, 128)], x2t_sb[:],
                    start=False, stop=True,
                )
                st = stp.tile([128, NS], bf16, name=f"st{rt}")
                nc.scalar.activation(
                    st[:], sps[:, :NS], AF.Exp, bias=ng_sb[:, rt : rt + 1],
                    scale=-1.0,
                )
                st_tiles.append(st)

            pending = [None]  # deferred softmax emitter for the previous n-tile

            for nt in range(NT):
                st_n = [st[:, ts(nt, 128)] for st in st_tiles]

                # den + bias consequent (shares the st_n stationaries with
                # the chunk matmuls that follow)
                dbp = psum.tile([128, CHUNK], f32, tag="bank2", name=f"dbp{nt}", bufs=4)
                nc.tensor.matmul(
                    dbp[:, : O + 2], st_n[0], cbo_sb[:, 0 : O + 2],
                    start=True, stop=False,
                )
                nc.tensor.matmul(
                    dbp[:, : O + 2], st_n[1], cbo_sb[:, O + 2 : 2 * (O + 2)],
                    start=False, stop=True,
                )
                denc = small.tile([128, 1], f32, name=f"denc{nt}")
                nc.vector.tensor_scalar_add(denc[:], dbp[:, :1], 1e-8)
                scalec = small.tile([128, 1], f32, name=f"scalec{nt}")
                nc.vector.reciprocal(scalec[:], denc[:])
                tb_sb = small.tile([128, O], f32, name=f"tb{nt}", tag="tb")
                nc.scalar.activation(tb_sb[:], dbp[:, 2 : O + 2], AF.Copy)

                # prod layout: [n, j, d] (d contiguous)
                prod = prodp.tile([128, O, D], bf16, name=f"prod{nt}", tag="prod")
                xrow = xn_sb[:, ts(nt, D)]  # [128 n, 128 d] bf16
                xb = xrow.unsqueeze(1).broadcast_to([128, JPC, D])

                tred = small.tile([128, O], f32, name=f"tred{nt}", tag="tred")
                if TREE_MODE == 'pyramid':
                    sbuf_s = small.tile(
                        [128, O, D // 2], bf16, tag="tree", name=f"s{nt}"
                    )

                def emit_tree(jlo, jhi):
                    if TREE_MODE == 'reduce':
                        nc.vector.tensor_reduce(
                            tred[:, jlo:jhi], prod[:, jlo:jhi, :],
                            axis=mybir.AxisListType.X, op=ALU.add,
                        )
                        return
                    sg = sbuf_s[:, jlo:jhi, :]
                    nc.vector.tensor_tensor(
                        sg, prod[:, jlo:jhi, 0 : D // 2],
                        prod[:, jlo:jhi, D // 2 : D], ALU.add,
                    )
                    h = D // 2
                    while h > 8:
                        h //= 2
                        nc.vector.tensor_tensor(
                            sg[:, :, 0:h], sg[:, :, 0:h], sg[:, :, h : 2 * h],
                            ALU.add,
                        )
                    nc.vector.tensor_reduce(
                        tred[:, jlo:jhi], sg[:, :, 0:8],
                        axis=mybir.AxisListType.X, op=ALU.add,
                    )

                for c in range(NCHUNK):
                    tps = psum.tile(
                        [128, CHUNK], f32, tag="bank2", name=f"tps{nt}_{c}", bufs=4
                    )
                    # rt-grouped emission: both 512-halves of the rt0 partial
                    # first (one stationary), then both rt1 halves.
                    for rt in range(RT):
                        for half in range(CHUNK // MM):
                            hsl = slice(half * MM, (half + 1) * MM)
                            base = c * CHUNK + half * MM
                            nc.tensor.matmul(
                                tps[:, hsl], st_n[rt],
                                c_sb[:, rt * DJ + base : rt * DJ + base + MM],
                                start=(rt == 0), stop=(rt == RT - 1),
                            )
                    oview = prod[:, c * JPC : (c + 1) * JPC, :]  # [128, 8, 128]
                    path = PATHS[nt][c]
                    if path == 'F':
                        tview = tps[:].rearrange("p (j d) -> p j d", j=JPC)
                        nc.vector.tensor_tensor(oview, tview, xb, ALU.mult)
                    else:
                        tcp = small.tile(
                            [128, JPC, D], bf16, tag="tcp", name=f"tcp{nt}_{c}",
                            bufs=4,
                        )
                        nc.scalar.activation(tcp[:], tps[:], AF.Copy)
                        eng = nc.gpsimd if path == 'P' else nc.vector
                        eng.tensor_tensor(oview, tcp[:], xb, ALU.mult)

                    if c == 1 and pending[0] is not None:
                        pending[0]()
                        pending[0] = None
                    if c == 5:
                        emit_tree(0, O // 2)
                    if c == NCHUNK - 1:
                        emit_tree(O // 2, O)

                # acc = tree + Tb
                acc = small.tile([128, O], f32, name=f"acc{nt}")
                nc.vector.scalar_tensor_tensor(
                    acc[:], tred[:], 1.0, tb_sb[:], ALU.mult, ALU.add
                )

                # softmax over j of logits = acc/den (no max-subtraction:
                # |logits| <= ~1.2 so exp cannot overflow)
                def make_softmax(nt=nt, acc=acc, scalec=scalec):
                    def emit():
                        exps = small.tile([128, O], f32, name=f"exps{nt}")
                        sume = small.tile([128, 1], f32, name=f"sume{nt}")
                        nc.scalar.activation(
                            exps[:], acc[:], AF.Exp, scale=scalec[:],
                            accum_out=sume[:],
                        )
                        rs = small.tile([128, 1], f32, name=f"rs{nt}")
                        nc.vector.reciprocal(rs[:], sume[:])
                        osb = small.tile([128, O], f32, name=f"osb{nt}")
                        nc.scalar.activation(osb[:], exps[:], AF.Copy, scale=rs[:])
                        nc.sync.dma_start(out_d[ts(nt, 128), :], osb[:])
                    return emit

                if nt == NT - 1:
                    make_softmax()()
                else:
                    pending[0] = make_softmax()

    nc.compile()
    return nc


def _prep_inputs(X, centers, sigmas, coeffs):
    """Host-side sharding + layout transforms (numpy only)."""
    X = np.ascontiguousarray(X, dtype=np.float32)
    centers = np.asarray(centers, dtype=np.float32)
    sigmas = np.asarray(sigmas, dtype=np.float32)
    coeffs = np.asarray(coeffs, dtype=np.float32)

    inv2s2 = 1.0 / (2.0 * sigmas * sigmas)            # [R, D]
    A = np.ascontiguousarray(inv2s2.T).astype(BF16)   # [D, R]
    B = np.ascontiguousarray((-centers / (sigmas * sigmas)).T).astype(BF16)  # [D, R]
    G = (centers * centers * inv2s2).sum(axis=1)      # [R]
    negG = np.ascontiguousarray(-G.reshape(RT, 128).T)  # [128, RT]

    # C in [r-low 128, rt, (j, d)] layout, bf16
    Cjd = np.ascontiguousarray(coeffs[:, :D, :].transpose(0, 2, 1))  # [R, O, D]
    Ck = np.ascontiguousarray(
        Cjd.reshape(RT, 128, DJ).transpose(1, 0, 2).reshape(128, RT * DJ)
    ).astype(BF16)
    Cb = coeffs[:, D, :].reshape(RT, 128, O).transpose(1, 0, 2)  # [128, RT, O]
    Cbo = np.ones((128, RT, O + 2), dtype=np.float32)
    Cbo[:, :, 2:] = Cb
    Cbo = np.ascontiguousarray(Cbo.reshape(128, RT * (O + 2))).astype(BF16)

    in_maps = []
    for i in range(NCORES):
        Xs = X[i * NS : (i + 1) * NS]                  # [512, 128]
        xt = np.ascontiguousarray(Xs.T).astype(BF16)   # [128, 512]
        x2t = np.ascontiguousarray(
            (Xs.T * Xs.T).astype(np.float32)
        ).astype(BF16)                                 # [128, 512]
        xn = np.ascontiguousarray(
            Xs.reshape(NT, 128, D).transpose(1, 0, 2).reshape(128, NT * D)
        ).astype(BF16)
        in_maps.append(
            {
                "xt": xt,
                "x2t": x2t,
                "xn": xn,
                "a_p": A,
                "b_p": B,
                "negg": negG,
                "cflat": Ck,
                "cbo": Cbo,
            }
        )
    return in_maps


def kernel(X, centers, sigmas, coeffs):
    from concourse.bass_utils import run_bass_kernel_spmd

    if "nc" not in _CACHE:
        _CACHE["nc"] = _build()
    nc = _CACHE["nc"]

    in_maps = _prep_inputs(X, centers, sigmas, coeffs)
    res = run_bass_kernel_spmd(nc, in_maps, list(range(NCORES)))
    out = np.concatenate([res.results[i]["out"] for i in range(NCORES)], axis=0)
    return out.astype(np.float32)


if __name__ == "__main__":
    rng = np.random.default_rng(0)
    X = rng.standard_normal((N, D), dtype=np.float32)
    centers = 0.5 * rng.standard_normal((R, D)).astype(np.float32)
    sigmas = (1.5 + rng.random((R, D))).astype(np.float32)
    coeffs = (0.02 * rng.standard_normal((R, D + 1, O))).astype(np.float32)
    out = kernel(X=X, centers=centers, sigmas=sigmas, coeffs=coeffs)
    print(out.shape, out.dtype, out.sum(axis=1)[:4])


# revision 16
# speedup vs baseline: 1.2030x; 1.2030x over previous
"""Trainium2 Bass kernel for nn_CustomANFIS (N=4096, D=128, R=256, O=64).

Math (reference):
  memb[n,r,d]  = exp(-(x[n,d]-c[r,d])^2 / (2 s[r,d]^2))
  str[n,r]     = prod_d memb = exp(-q[n,r]) with
                 q[n,r] = sum_d x^2[n,d]*A[d,r] + sum_d x[n,d]*B[d,r] + G[r],
                 A = 1/(2 s^2), B = -c/s^2, G = sum_d c^2/(2 s^2)
  den[n]       = sum_r str + 1e-8
  W[n,r,:]     = x[n,:] @ coeffs[r,:D,:] + coeffs[r,D,:]
  out          = softmax_j( (1/den) * sum_r str[n,r] * W[n,r,j] )

Device algorithm (data-parallel over N across 8 cores), v6:
  * strengths^T [r, n] via bf16 accumulating matmuls + ACT exp (bias=-G).
  * per n-tile: den/bias matmul against [1|1|Cb], then 8 chunks of
    T[n, ...]; chunk c carries d-block c (16 d) for ALL 64 j (j-major
    inside), so prod is [128, 8 dblk, 64 j, 16 dw] and every
    d-reduction level is a CONTIGUOUS whole-block add.
  * consumer: 'A' chunks ACT-copy PSUM->SBUF bf16 then DVE multiply by
    x (2x packed); 'F' chunks DVE-multiply straight from PSUM. GpSimd
    compute is never used (its SBUF port contends with the DVE and
    halves DVE throughput while active).
  * d-reduction: level 1 = four DISJOINT block-pair RMW adds issued as
    gpsimd accumulating DMAs on the otherwise-idle SDMA engines
    (~1.9us each, 4 fit in one n-tile window); levels 2/3 + the 16->1
    tail + softmax run on the DVE/ACT but are deferred up to TWO
    n-tiles later, giving multi-microsecond margins around the RMW
    completion (its semaphore fires before the write fully lands).
    The last n-tile keeps everything on the DVE for a short tail.
  * softmax without max-subtraction (|logits| <= ~1.2).

Head: ~5.7us NEFF preamble is fixed; DMA-completion waits coalesce per
queue, so compute is emitted before any DMA it does not need: two
packed bf16 input tensors ([xt|b] on sync, [x2t|a|xn|cbo] on scalar)
+ ng go first, strengths matmuls next, and the 8 C-chunk DMAs are
interleaved into the first n-tile's matmul emission.
"""

import numpy as np
import ml_dtypes

N, D, R, O = 4096, 128, 256, 64
NCORES = 8
NS = N // NCORES          # 512 rows per core
NT = NS // 128            # 4 n-tiles per core
RT = R // 128             # 2 r k-tiles
DJ = D * O                # 8192
CHUNK = 1024              # 2 PSUM banks per chunk
NCHUNK = DJ // CHUNK      # 8 chunks (16 d x 64 j each)
DW = D // NCHUNK          # 16 d per chunk
MM = 512                  # moving free dim per matmul

# packed inputs (bf16): packa = [xt 512 | b 256], packb = [x2t 512 | a 256 | xn 512 | cbo 132]
PA_XT, PA_B, PAW = 0, 512, 768
PB_X2T, PB_A, PB_XN, PB_CBO = 0, 512, 768, 1280
PBW = 1280 + RT * (O + 2)  # 1412

# per-chunk consumer path: 'A' = ACT copy + DVE mult, 'F' = DVE from PSUM
PATHS = (
    ('A', 'A', 'A', 'A', 'A', 'A', 'F', 'A'),
    ('A', 'A', 'A', 'A', 'A', 'A', 'F', 'A'),
    ('A', 'A', 'A', 'A', 'A', 'A', 'F', 'A'),
    ('A', 'A', 'A', 'A', 'A', 'F', 'F', 'F'),
)
DMA_TREE = False          # gpsimd accum-DMA completion sems fire at
                          # descriptor-gen, not write-landed: racy. Keep off.

_CACHE = {}
BF16 = ml_dtypes.bfloat16


def _build():
    import concourse.bass as bass
    import concourse.tile as tile
    from concourse import bacc, mybir

    f32 = mybir.dt.float32
    bf16 = mybir.dt.bfloat16
    AF = mybir.ActivationFunctionType
    ALU = mybir.AluOpType
    ts = bass.ts

    nc = bacc.Bacc(
        "TRN2", target_bir_lowering=False, debug=False, num_devices=NCORES
    )

    pa_d = nc.dram_tensor("packa", [128, PAW], bf16, kind="ExternalInput").ap()
    pb_d = nc.dram_tensor("packb", [128, PBW], bf16, kind="ExternalInput").ap()
    ng_d = nc.dram_tensor("negg", [128, RT], f32, kind="ExternalInput").ap()
    c_d = nc.dram_tensor("cflat", [128, RT * DJ], bf16, kind="ExternalInput").ap()
    out_d = nc.dram_tensor("out", [NS, O], f32, kind="ExternalOutput").ap()

    with tile.TileContext(nc) as tc:
        from contextlib import ExitStack

        with ExitStack() as ctx:
            konst = ctx.enter_context(tc.tile_pool(name="konst", bufs=1))
            cw = ctx.enter_context(tc.tile_pool(name="cw", bufs=1))
            stp = ctx.enter_context(tc.tile_pool(name="stp", bufs=1))
            prodp = ctx.enter_context(tc.tile_pool(name="prodp", bufs=3))
            small = ctx.enter_context(tc.tile_pool(name="small", bufs=8))
            psum = ctx.enter_context(tc.tile_pool(name="psum", bufs=2, space="PSUM"))

            pa_sb = konst.tile([128, PAW], bf16)
            pb_sb = konst.tile([128, PBW], bf16)
            ng_sb = konst.tile([128, RT], f32)
            c_sb = cw.tile([128, RT * DJ], bf16)

            xt_sb = pa_sb[:, PA_XT : PA_XT + NS]
            b_sb = pa_sb[:, PA_B : PA_B + R]
            x2t_sb = pb_sb[:, PB_X2T : PB_X2T + NS]
            a_sb = pb_sb[:, PB_A : PB_A + R]
            xn_sb = pb_sb[:, PB_XN : PB_XN + NT * D]
            cbo_sb = pb_sb[:, PB_CBO : PB_CBO + RT * (O + 2)]

            c_v = c_sb[:].rearrange("p (t f) -> p t f", t=RT)
            cd_v = c_d.rearrange("p (t f) -> p t f", t=RT)

            def c_dma(eng, c):
                csl = slice(c * CHUNK, (c + 1) * CHUNK)
                eng.dma_start(c_v[:, :, csl], cd_v[:, :, csl])

            # critical inputs first: every DMA on a queue gates all compute
            # emitted after it (lane-coalesced waits).
            nc.sync.dma_start(pa_sb[:], pa_d)
            nc.scalar.dma_start(ng_sb[:], ng_d)
            nc.scalar.dma_start(pb_sb[:], pb_d)

            # warm the ACT exp table set (~2.7us) during the DMA head
            dummy = konst.tile([128, 1], f32, name="dummy")
            nc.vector.memset(dummy[:], 0.0)
            dummy2 = konst.tile([128, 1], f32, name="dummy2")
            nc.scalar.activation(dummy2[:], dummy[:], AF.Exp)

            # warm the gpsimd SWDGE path (first-run ring/ucode init costs
            # several us and would otherwise delay n-tile 0's tree RMWs)
            bar = konst.tile([128, 16], bf16, name="bar")
            barz = konst.tile([128, 16], bf16, name="barz")
            nc.vector.memset(barz[:], 0.0)
            nc.gpsimd.dma_start(bar[:], barz[:], accum_op=mybir.AluOpType.bypass)

            # ---- strengths^T: [r-tile partitions, n free], bf16
            st_tiles = []
            for rt in range(RT):
                sps = psum.tile([128, CHUNK], f32, tag="bank2", name=f"sps{rt}", bufs=4)
                nc.tensor.matmul(
                    sps[:, :NS], b_sb[:, ts(rt, 128)], xt_sb,
                    start=True, stop=False,
                )
                nc.tensor.matmul(
                    sps[:, :NS], a_sb[:, ts(rt, 128)], x2t_sb,
                    start=False, stop=True,
                )
                st = stp.tile([128, NS], bf16, name=f"st{rt}")
                nc.scalar.activation(
                    st[:], sps[:, :NS], AF.Exp, bias=ng_sb[:, rt : rt + 1],
                    scale=-1.0,
                )
                st_tiles.append(st)

            c_dma(nc.sync, 0)
            c_dma(nc.scalar, 1)
            c_queue = {2: nc.sync, 3: nc.scalar, 4: nc.sync, 5: nc.scalar,
                       6: nc.sync, 7: nc.scalar}

            # deferred ladders: prevs[0] = n-tile i-1, prevs[1] = n-tile i-2
            prevs = [None, None]

            for nt in range(NT):
                st_n = [st[:, ts(nt, 128)] for st in st_tiles]
                last = nt == NT - 1

                dbp = psum.tile([128, CHUNK], f32, tag="bank2", name=f"dbp{nt}", bufs=4)
                nc.tensor.matmul(
                    dbp[:, : O + 2], st_n[0], cbo_sb[:, 0 : O + 2],
                    start=True, stop=False,
                )
                nc.tensor.matmul(
                    dbp[:, : O + 2], st_n[1], cbo_sb[:, O + 2 : 2 * (O + 2)],
                    start=False, stop=True,
                )
                denc = small.tile([128, 1], f32, name=f"denc{nt}")
                nc.vector.tensor_scalar_add(denc[:], dbp[:, :1], 1e-8)
                scalec = small.tile([128, 1], f32, name=f"scalec{nt}")
                nc.vector.reciprocal(scalec[:], denc[:])
                tb_sb = small.tile([128, O], f32, name=f"tb{nt}", tag="tb", bufs=4)
                nc.scalar.activation(tb_sb[:], dbp[:, 2 : O + 2], AF.Copy)

                # prod layout: [n, dblk, j, dw] (whole d-blocks contiguous)
                prod = prodp.tile([128, NCHUNK, O, DW], bf16,
                                  name=f"prod{nt}", tag="prod")
                xrow = xn_sb[:, ts(nt, D)]  # [128 n, 128 d] bf16

                def make_ladder(nt=nt, prod=prod, tb_sb=tb_sb, scalec=scalec):
                    state = {}
                    def l2():
                        nc.vector.tensor_tensor(prod[:, 0], prod[:, 0],
                                                prod[:, 2], ALU.add)
                        nc.vector.tensor_tensor(prod[:, 4], prod[:, 4],
                                                prod[:, 6], ALU.add)
                    def l3():
                        nc.vector.tensor_tensor(prod[:, 0], prod[:, 0],
                                                prod[:, 4], ALU.add)
                    def vt():
                        nc.vector.tensor_tensor(
                            prod[:, 0, :, 0:8], prod[:, 0, :, 0:8],
                            prod[:, 0, :, 8:16], ALU.add,
                        )
                        nc.vector.tensor_tensor(
                            prod[:, 0, :, 0:4], prod[:, 0, :, 0:4],
                            prod[:, 0, :, 4:8], ALU.add,
                        )
                        tred = small.tile([128, O], f32, name=f"tred{nt}",
                                          tag="tred", bufs=4)
                        nc.vector.tensor_reduce(
                            tred[:], prod[:, 0, :, 0:4],
                            axis=mybir.AxisListType.X, op=ALU.add,
                        )
                        acc = small.tile([128, O], f32, name=f"acc{nt}")
                        nc.vector.scalar_tensor_tensor(
                            acc[:], tred[:], 1.0, tb_sb[:], ALU.mult, ALU.add
                        )
                        state['acc'] = acc
                    def soft():
                        acc = state['acc']
                        exps = small.tile([128, O], f32, name=f"exps{nt}")
                        sume = small.tile([128, 1], f32, name=f"sume{nt}")
                        nc.scalar.activation(
                            exps[:], acc[:], AF.Exp, scale=scalec[:],
                            accum_out=sume[:],
                        )
                        rs = small.tile([128, 1], f32, name=f"rs{nt}")
                        nc.vector.reciprocal(rs[:], sume[:])
                        osb = small.tile([128, O], f32, name=f"osb{nt}")
                        nc.scalar.activation(osb[:], exps[:], AF.Copy, scale=rs[:])
                        nc.sync.dma_start(out_d[ts(nt, 128), :], osb[:])
                    return {'l2': l2, 'l3': l3, 'vt': vt, 'soft': soft}

                for c in range(NCHUNK):
                    tps = psum.tile(
                        [128, CHUNK], f32, tag="bank2", name=f"tps{nt}_{c}", bufs=4
                    )
                    for rt in range(RT):
                        for half in range(CHUNK // MM):
                            hsl = slice(half * MM, (half + 1) * MM)
                            base = c * CHUNK + half * MM
                            nc.tensor.matmul(
                                tps[:, hsl], st_n[rt],
                                c_sb[:, rt * DJ + base : rt * DJ + base + MM],
                                start=(rt == 0), stop=(rt == RT - 1),
                            )
                    if nt == 0 and c in c_queue:
                        c_dma(c_queue[c], c)

                    oview = prod[:, c]  # [128, 64, 16]
                    xb = (xrow[:, c * DW : (c + 1) * DW]
                          .unsqueeze(1).broadcast_to([128, O, DW]))
                    if PATHS[nt][c] == 'F':
                        tview = tps[:].rearrange("p (j w) -> p j w", j=O)
                        nc.vector.tensor_tensor(oview, tview, xb, ALU.mult)
                    else:
                        tcp = small.tile(
                            [128, O, DW], bf16, tag="tcp", name=f"tcp{nt}_{c}",
                            bufs=4,
                        )
                        nc.scalar.activation(tcp[:], tps[:], AF.Copy)
                        nc.vector.tensor_tensor(oview, tcp[:], xb, ALU.mult)

                    # deferred ladders: i-2 early, i-1 late in this n-tile
                    if c == 1 and prevs[1] is not None:
                        prevs[1]['vt']()
                    elif c == 3 and prevs[1] is not None:
                        prevs[1]['soft']()
                    elif c == 5 and prevs[0] is not None:
                        prevs[0]['l2']()
                    elif c == 7 and prevs[0] is not None:
                        prevs[0]['l3']()

                    # this n-tile's level-1 pair adds (disjoint RMWs)
                    if c % 2 == 1:
                        if not last and DMA_TREE:
                            nc.gpsimd.dma_start(prod[:, c - 1], prod[:, c],
                                                accum_op=ALU.add)
                            if c == NCHUNK - 1:
                                # barrier: a plain DMA on the same ring drains
                                # after every pair-RMW descriptor per engine,
                                # and ITS completion semaphore is reliable —
                                # the accum-DMA sem can fire before the RMW
                                # write fully lands.
                                nc.gpsimd.dma_start(bar[:], prod[:, 0, 0, :])
                        else:
                            nc.vector.tensor_tensor(prod[:, c - 1], prod[:, c - 1],
                                                    prod[:, c], ALU.add)

                lad = make_ladder()
                if last:
                    # flush: i-2 (=nt1) was handled in-loop; finish nt2 + nt3
                    if prevs[0] is not None:
                        prevs[0]['vt']()
                        prevs[0]['soft']()
                    lad['l2']()
                    lad['l3']()
                    lad['vt']()
                    lad['soft']()
                else:
                    prevs[1] = prevs[0]
                    prevs[0] = lad

    nc.compile()
    return nc


def _prep_inputs(X, centers, sigmas, coeffs):
    """Host-side sharding + layout transforms (numpy only)."""
    X = np.ascontiguousarray(X, dtype=np.float32)
    centers = np.asarray(centers, dtype=np.float32)
    sigmas = np.asarray(sigmas, dtype=np.float32)
    coeffs = np.asarray(coeffs, dtype=np.float32)

    inv2s2 = 1.0 / (2.0 * sigmas * sigmas)            # [R, D]
    A = np.ascontiguousarray(inv2s2.T)                # [D, R]
    B = np.ascontiguousarray((-centers / (sigmas * sigmas)).T)  # [D, R]
    G = (centers * centers * inv2s2).sum(axis=1)      # [R]
    negG = np.ascontiguousarray(-G.reshape(RT, 128).T)  # [128, RT]

    # C chunks carry one 16-wide d-block for all j (j-major inside)
    Cjd = coeffs[:, :D, :].transpose(0, 2, 1)          # [R, O, D]
    Cblk = Cjd.reshape(R, O, NCHUNK, DW).transpose(0, 2, 1, 3)  # [R, blk, O, dw]
    Ck = np.ascontiguousarray(
        Cblk.reshape(RT, 128, DJ).transpose(1, 0, 2).reshape(128, RT * DJ)
    ).astype(BF16)
    Cb = coeffs[:, D, :].reshape(RT, 128, O).transpose(1, 0, 2)  # [128, RT, O]
    Cbo = np.ones((128, RT, O + 2), dtype=np.float32)
    Cbo[:, :, 2:] = Cb
    Cbo = Cbo.reshape(128, RT * (O + 2))

    in_maps = []
    for i in range(NCORES):
        Xs = X[i * NS : (i + 1) * NS]                  # [512, 128]
        xt = Xs.T                                      # [128, 512]
        xn = np.ascontiguousarray(
            Xs.reshape(NT, 128, D).transpose(1, 0, 2).reshape(128, NT * D)
        )
        pa = np.empty((128, PAW), dtype=np.float32)
        pa[:, PA_XT : PA_XT + NS] = xt
        pa[:, PA_B : PA_B + R] = B
        pb = np.empty((128, PBW), dtype=np.float32)
        pb[:, PB_X2T : PB_X2T + NS] = xt * xt
        pb[:, PB_A : PB_A + R] = A
        pb[:, PB_XN : PB_XN + NT * D] = xn
        pb[:, PB_CBO : PB_CBO + RT * (O + 2)] = Cbo
        in_maps.append(
            {
                "packa": np.ascontiguousarray(pa).astype(BF16),
                "packb": np.ascontiguousarray(pb).astype(BF16),
                "negg": negG,
                "cflat": Ck,
            }
        )
    return in_maps


def kernel(X, centers, sigmas, coeffs):
    from concourse.bass_utils import run_bass_kernel_spmd

    if "nc" not in _CACHE:
        _CACHE["nc"] = _build()
    nc = _CACHE["nc"]

    in_maps = _prep_inputs(X, centers, sigmas, coeffs)
    res = run_bass_kernel_spmd(nc, in_maps, list(range(NCORES)))
    out = np.concatenate([res.results[i]["out"] for i in range(NCORES)], axis=0)
    return out.astype(np.float32)


if __name__ == "__main__":
    rng = np.random.default_rng(0)
    X = rng.standard_normal((N, D), dtype=np.float32)
    centers = 0.5 * rng.standard_normal((R, D)).astype(np.float32)
    sigmas = (1.5 + rng.random((R, D))).astype(np.float32)
    coeffs = (0.02 * rng.standard_normal((R, D + 1, O))).astype(np.float32)
    out = kernel(X=X, centers=centers, sigmas=sigmas, coeffs=coeffs)
    print(out.shape, out.dtype, out.sum(axis=1)[:4])
